# revision 28
# baseline (speedup 1.0000x reference)
"""Causal attention kernel for Trainium2 (Bass/Tile), 8-core SPMD.

Problem: out = softmax(causal(C @ B^T / sqrt(64))) @ x
  x, B, C: [2, 8, 4096, 64] fp32  (V, K, Q respectively)

Sharding: batch*heads = 16 slices -> 2 per core across 8 cores.

Per-head algorithm (L=4096, D=64, i=query tile, j=key tile, j<=i):
  - Q^T/K^T staged as bf16 [64, L] via PE transposes (bf16 converts on
    GpSimd, copies on DVE/ACT); V as bf16 with a ones-column appended
    so the PV matmul also accumulates the softmax denominator.
  - Only the first Q/K batch + V of head 0 stage upfront; all other
    batches stream through the attention loop (DMA+convert issued ~6
    groups before their PE transposes, which borrow score-ring PSUM
    slots), so staging overlaps compute.
  - 4 passes per head, each covering 1024 queries (i-tiles [8p, 8p+8)).
    O^T accumulates in 2 PSUM banks [65, 512] per pass.
  - Per (pass, j) one "group": S^T[kk, q] = K_j @ Q^T over the pass's
    1024 q, PSUM f32 (2 banks, 3-deep ring => 2-group lookahead).
  - exp(score/8): ScalarE (exact, bf16 out) for most groups, then a DVE
    multiply with a 0/1 lower-triangle mask on the diagonal tile; every
    DVE_STRIDE-th full off-diagonal group instead computes exp on DVE
    via a bf16 Schraudolph bitcast fast-exp (int16(s*FA+FB) bits), to
    unload ScalarE.  DVE-group exps are emitted at QK time so they
    overlap the lookahead window.
  - Emission is software-pipelined with 2-group lookahead: PE order is
    QK(n), PV(n-2) so the PE never stalls on ScalarE latency.
  - Epilogue per bank ("quarter"): emitted right after that bank's last
    PV: copy O^T to SBUF (DVE), 4 PE transposes into one PSUM slot, one
    reciprocal + one broadcast multiply, single 4-tile DMA out.  Bank0
    quarters land mid-pass, so epilogue work overlaps attention.
"""

import os
from contextlib import ExitStack

import numpy as np

L = 4096
D = 64
P = 128
NT = L // P           # 32 query/key tiles per head
PASS_T = 8            # i-tiles per pass
N_PASS = NT // PASS_T
HEADS_PER_CORE = 2
N_CORES = 8

# Every DVE_STRIDE-th eligible (full, off-diagonal) group computes exp on
# DVE via fast-exp instead of ScalarE. 0 disables.
DVE_STRIDE = int(os.environ.get("KERNEL_DVE_STRIDE", "8"))

# Schraudolph fast-exp in bf16: exp(s*SCALE) ~= bf16_bits(int16(s*FA + FB)).
# FB tuned for truncation rounding; max rel err ~3.3e-2, end-to-end ~7e-3.
SCALE = 0.125
FA = float((1 << 7) / np.log(2.0) * SCALE)
FB = float(127.0 * (1 << 7) - 5.0)

_cache = {}


def _build_nc():
    import concourse.mybir as mybir
    import concourse.tile as tile
    from concourse import bacc
    from concourse.masks import make_identity

    f32 = mybir.dt.float32
    bf16 = mybir.dt.bfloat16
    i16 = mybir.dt.int16
    EXP = mybir.ActivationFunctionType.Exp

    nc = bacc.Bacc("TRN2", target_bir_lowering=False, debug=False)

    x_t = nc.dram_tensor("x", (HEADS_PER_CORE, L, D), f32, kind="ExternalInput")
    b_t = nc.dram_tensor("B", (HEADS_PER_CORE, L, D), f32, kind="ExternalInput")
    c_t = nc.dram_tensor("C", (HEADS_PER_CORE, L, D), f32, kind="ExternalInput")
    o_t = nc.dram_tensor("out", (HEADS_PER_CORE, L, D), f32, kind="ExternalOutput")
    x_ap, b_ap, c_ap, o_ap = x_t.ap(), b_t.ap(), c_t.ap(), o_t.ap()

    with tile.TileContext(nc) as tc, ExitStack() as ctx:
        const = ctx.enter_context(tc.tile_pool(name="const", bufs=1))
        identity = const.tile([P, P], f32)
        make_identity(nc, identity[:])
        ident_b = const.tile([P, P], bf16)
        nc.vector.tensor_copy(ident_b[:], identity[:])
        # dmask[kk, q] = 1 where kk <= q else 0 (S^T coords, keep-lower).
        dmask = const.tile([P, P], bf16)
        nc.gpsimd.memset(dmask[:], 1.0)
        nc.gpsimd.affine_select(
            out=dmask[:],
            in_=dmask[:],
            compare_op=mybir.AluOpType.is_ge,
            fill=0.0,
            base=0,
            pattern=[[1, P]],       # +q
            channel_multiplier=-1,  # -kk  => keep where q - kk >= 0
        )

        qkv = ctx.enter_context(tc.tile_pool(name="qkv", bufs=1))
        qt = {}
        kt = {}
        v1b = {}

        # ---- Stage inputs: Q^T/K^T (bf16) upfront; V per-head (h1 V is
        # staged later, interleaved into h0's attention: DMA+DVE only) ----
        stage = ctx.enter_context(tc.tile_pool(name="stage", bufs=3))

        BT = 8  # i-tiles per DMA/transpose batch

        for h in range(HEADS_PER_CORE):
            v1b[h] = qkv.tile([P, NT, D + 1], bf16, name=f"v1b_{h}", tag=f"v1b_{h}")
            for nm in ("q", "k"):
                dm = qt if nm == "q" else kt
                dm[h] = qkv.tile(
                    [D, NT, P], bf16, name=f"{nm}t_{h}", tag=f"{nm}t_{h}"
                )

        def stage_dma_convert(h, g, nm, bt=None, t0=None, conv="pool"):
            bt = bt or BT
            t0 = g * BT if t0 is None else t0
            src_ap = c_ap if nm == "q" else b_ap
            st = stage.tile([P, bt, D], f32, name="st", tag="stage_in")
            nc.sync.dma_start(
                out=st[:],
                in_=src_ap[h, t0 * P : (t0 + bt) * P].rearrange(
                    "(a p) d -> p a d", p=P
                ),
            )
            stb = stage.tile([P, bt, D], bf16, name="stb", tag="stage_b")
            if conv == "dve":
                nc.vector.tensor_copy(stb[:], st[:])
            else:
                nc.gpsimd.tensor_copy(stb[:], st[:])
            return stb

        def stage_transpose_copy(h, g, nm, stb, pt_pool, pt_tag, copy_eng,
                                 bt=None, t0=None):
            bt = bt or BT
            t0 = g * BT if t0 is None else t0
            dst = (qt if nm == "q" else kt)[h]
            pt = pt_pool.tile([D, bt, P], bf16, name="pt", tag=pt_tag)
            for a in range(bt):
                nc.tensor.transpose(pt[:, a], stb[:, a], ident_b[:])
            if copy_eng == "act":
                nc.scalar.copy(dst[:, t0 : t0 + bt], pt[:])
            else:
                nc.vector.tensor_copy(dst[:, t0 : t0 + bt], pt[:])

        def emit_v_stage(h):
            # 4 chunked DMAs + copies so the first j-tiles are ready early
            NC4 = NT // 4
            for c in range(4):
                vs = stage.tile([P, NC4, D], f32, name="vs4", tag="vstage")
                nc.sync.dma_start(
                    out=vs[:],
                    in_=x_ap[h, c * NC4 * P : (c + 1) * NC4 * P].rearrange(
                        "(j p) d -> p j d", p=P
                    ),
                )
                nc.vector.tensor_copy(
                    v1b[h][:, c * NC4 : (c + 1) * NC4, 0:D], vs[:]
                )
            nc.vector.memset(v1b[h][:, :, D], 1.0)

        # Upfront: only the first batch of Q/K for head 0 (PE transposes in
        # a dedicated PSUM pool) + head 0's V.  Everything else is staged
        # through the attention loop: DMA+convert issued ~6 groups before
        # the PE transposes, which borrow score-ring PSUM slots.
        with tc.tile_pool(name="tpsum", bufs=2, space="PSUM") as tpsum:
            for sub in range(2):  # 4-tile sub-batches: first transposes sooner
                for nm in ("q", "k"):
                    # f32 transpose straight off the DMA: drops the bf16
                    # convert from the startup critical chain
                    src_ap = c_ap if nm == "q" else b_ap
                    t0 = sub * 4
                    st = stage.tile([P, 4, D], f32, name="st", tag="stage_in")
                    nc.sync.dma_start(
                        out=st[:],
                        in_=src_ap[0, t0 * P : (t0 + 4) * P].rearrange(
                            "(a p) d -> p a d", p=P
                        ),
                    )
                    ptf = tpsum.tile([D, 4, P], f32, name="ptf", tag="tp")
                    for a in range(4):
                        nc.tensor.transpose(ptf[:, a], st[:, a], identity[:])
                    dst = (qt if nm == "q" else kt)[0]
                    if nm == "k":
                        nc.scalar.copy(dst[:, t0 : t0 + 4], ptf[:])
                    else:
                        nc.vector.tensor_copy(dst[:, t0 : t0 + 4], ptf[:])
            emit_v_stage(0)

        # ---- Build the flat group list (4 passes x 2 heads) ----
        # Group = one j-tile within one pass: a [128, 1024] score tile.
        segments = []
        flat = []
        for h in range(HEADS_PER_CORE):
            for p in range(N_PASS):
                base = p * PASS_T
                seg = dict(h=h, base=base, idx=len(segments))
                segments.append(seg)
                for j in range(base + PASS_T):
                    q0l = (max(j, base) - base) * P
                    grp = dict(
                        h=h, base=base, j=j, q0l=q0l,
                        diag=j >= base, seg=seg["idx"],
                    )
                    flat.append(grp)

        elig = 0
        for grp in flat:
            grp["dve"] = False
            if not grp["diag"] and DVE_STRIDE:
                if elig % DVE_STRIDE == (DVE_STRIDE - 1):
                    grp["dve"] = True
                elig += 1

        with (
            tc.tile_pool(name="score", bufs=3, space="PSUM") as score_pool,
            tc.tile_pool(name="oacc", bufs=2, space="PSUM") as oacc_pool,
            tc.tile_pool(name="exps", bufs=6) as exps_pool,
            tc.tile_pool(name="epi", bufs=3) as epi_pool,
        ):
            obanks = {}   # seg idx -> [bank0, bank1] PSUM tiles

            def emit_qk(grp):
                h, j, q0l = grp["h"], grp["j"], grp["q0l"]
                base = grp["base"]
                sc = score_pool.tile([P, 1024], f32, name="sc", tag="score")
                grp["sc"] = sc
                kslice = kt[h][:, j]
                for b2 in range(2):
                    bs = b2 * 512
                    cs = max(q0l, bs)
                    w = bs + 512 - cs
                    if w <= 0:
                        continue
                    t0 = base + cs // P
                    nc.tensor.matmul(
                        sc[:, cs : cs + w],
                        lhsT=kslice,
                        rhs=qt[h][:, t0 : t0 + w // P],
                        start=True,
                        stop=True,
                    )

            def emit_exp(grp):
                q0l = grp["q0l"]
                sc = grp["sc"]
                if grp["dve"]:
                    et = exps_pool.tile([P, 1024], bf16, name="et", tag="exps")
                    nc.vector.tensor_scalar(
                        out=et[:].bitcast(i16),
                        in0=sc[:],
                        scalar1=FA,
                        scalar2=FB,
                        op0=mybir.AluOpType.mult,
                        op1=mybir.AluOpType.add,
                    )
                else:
                    et = exps_pool.tile([P, 1024], bf16, name="et", tag="exps")
                    nc.scalar.activation(
                        et[:, q0l:], sc[:, q0l:], EXP, scale=SCALE,
                    )
                    if grp["diag"]:
                        nc.vector.tensor_mul(
                            et[:, q0l : q0l + P], et[:, q0l : q0l + P], dmask[:]
                        )
                grp["et"] = et

            def emit_pv(grp):
                h, j, q0l = grp["h"], grp["j"], grp["q0l"]
                base, si = grp["base"], grp["seg"]
                if si not in obanks:
                    obanks[si] = [
                        oacc_pool.tile([D + 1, 512], f32, name="obank", tag="oacc")
                        for _ in range(2)
                    ]
                et = grp["et"]
                vsl = v1b[h][:, j]
                for b2 in range(2):
                    bs = b2 * 512
                    cs = max(q0l, bs)
                    w = bs + 512 - cs
                    if w <= 0:
                        continue
                    nc.tensor.matmul(
                        obanks[si][b2][:, cs - bs : cs - bs + w],
                        lhsT=vsl,
                        rhs=et[:, cs : cs + w],
                        start=(j == 0),
                        stop=(j == base + 4 * b2 + 3),
                    )

            def emit_quarter(si, bank, a0=0, na=4):
                # Drain + normalize + store `na` i-tiles of one O^T bank.
                seg = segments[si]
                h, base = seg["h"], seg["base"]
                osb = epi_pool.tile([D + 1, na * P], f32, name="osb", tag="osb")
                nc.vector.tensor_copy(
                    osb[:], obanks[si][bank][:, a0 * P : (a0 + na) * P]
                )
                tptn = oacc_pool.tile([P, na, D + 1], f32, name="tptn", tag="oacc")
                for a in range(na):
                    nc.tensor.transpose(
                        tptn[:, a],
                        osb[:, a * P : (a + 1) * P],
                        identity[: D + 1, : D + 1],
                    )
                rec = epi_pool.tile([P, na], f32, name="rec", tag="rec")
                nc.vector.reciprocal(rec[:], tptn[:, :, D])
                otn = epi_pool.tile([P, na, D], f32, name="otn", tag="ot")
                nc.vector.scalar_tensor_tensor(
                    out=otn[:],
                    in0=tptn[:, :, 0:D],
                    scalar=1.0,
                    in1=rec[:].broadcast_to((P, na, D)),
                    op0=mybir.AluOpType.mult,
                    op1=mybir.AluOpType.mult,
                )
                i0 = base + 4 * bank + a0
                nc.sync.dma_start(
                    out=o_ap[h, i0 * P : (i0 + na) * P].rearrange(
                        "(a p) d -> p a d", p=P
                    ),
                    in_=otn[:],
                )

            LOOKAHEAD = 2
            N = len(flat)
            # (transpose_at, h, g, nm): q batches ahead of the pass that
            # reads them as matmul rhs; k batches before their j-range.
            batch_plan = [
                (8, 0, 1, "q"), (14, 0, 1, "k"),
                (20, 0, 2, "q"), (28, 0, 2, "k"),
                (36, 0, 3, "q"), (44, 0, 3, "k"),
                (50, 1, 0, "q"), (54, 1, 0, "k"),
                (58, 1, 1, "q"), (62, 1, 1, "k"),
                (66, 1, 2, "q"), (70, 1, 2, "k"),
                (74, 1, 3, "q"), (77, 1, 3, "k"),
            ]
            dma_at = {n - 6: (h, g, nm) for n, h, g, nm in batch_plan}
            tp_at = {n: (h, g, nm) for n, h, g, nm in batch_plan}
            pending_stb = {}
            for n in range(N + LOOKAHEAD):
                if n in dma_at:
                    h, g, nm = dma_at[n]
                    pending_stb[(h, g, nm)] = stage_dma_convert(h, g, nm)
                if n in tp_at:
                    h, g, nm = tp_at[n]
                    stage_transpose_copy(
                        h, g, nm, pending_stb.pop((h, g, nm)),
                        score_pool, "score", "dve",
                    )
                if n == 40:
                    emit_v_stage(1)
                if n < N:
                    emit_qk(flat[n])
                    if flat[n]["dve"]:
                        # enqueue DVE fast-exp immediately so it overlaps
                        # the next two groups instead of stalling PV
                        emit_exp(flat[n])
                k = n - LOOKAHEAD
                if k >= 0:
                    grp = flat[k]
                    if not grp["dve"]:
                        emit_exp(grp)
                    emit_pv(grp)
                    # quarters fire when their bank took its last PV
                    for bank in range(2):
                        if grp["j"] == grp["base"] + 4 * bank + 3:
                            emit_quarter(grp["seg"], bank)

    nc.compile()
    return nc


def _get_nc():
    if "nc" not in _cache:
        _cache["nc"] = _build_nc()
    return _cache["nc"]


def kernel(x: np.ndarray, B: np.ndarray, C: np.ndarray) -> np.ndarray:
    from concourse import bass_utils

    BATCH, H = x.shape[0], x.shape[1]
    nbh = BATCH * H
    xf = np.ascontiguousarray(x.reshape(nbh, L, D), dtype=np.float32)
    bf = np.ascontiguousarray(B.reshape(nbh, L, D), dtype=np.float32)
    cf = np.ascontiguousarray(C.reshape(nbh, L, D), dtype=np.float32)

    nc = _get_nc()
    in_maps = []
    for c in range(N_CORES):
        s = slice(c * HEADS_PER_CORE, (c + 1) * HEADS_PER_CORE)
        in_maps.append(
            {
                "x": np.ascontiguousarray(xf[s]),
                "B": np.ascontiguousarray(bf[s]),
                "C": np.ascontiguousarray(cf[s]),
            }
        )

    res = bass_utils.run_bass_kernel_spmd(
        nc,
        in_maps,
        core_ids=list(range(N_CORES)),
        trace=False,
    )
    _cache["last_result"] = res

    out = np.empty((nbh, L, D), dtype=np.float32)
    for c in range(N_CORES):
        out[c * HEADS_PER_CORE : (c + 1) * HEADS_PER_CORE] = res.results[c]["out"]
    return out.reshape(BATCH, H, L, D)
